# revision 28
# baseline (speedup 1.0000x reference)
"""ClusterAttention Trainium2 kernel.

Problem: B=4, N=8192, C=384, H=12, D=2, K=256 clusters of M=32 members.
  qkv = feat @ qkv_w.T + qkv_b
  kv/pos gathered per cluster -> mean -> key/value per (batch, cluster, head)
  attn = softmax(q.k*scale + pos_bias) over clusters; out = attn@v; proj.

Key algebraic restructurings:
  1. mean-of-gather commutes with the linear kv projection: cluster kv means
     are computed as (feat^T @ S) @ W_kv.T where S is the one-hot cluster
     assignment matrix -- no gather of the (much larger) kv tensor is needed.
  2. pos_bias[b,h,n,k] = pos_mean[b,k]@pos_w[h] - pos[b,n]@pos_w[h] + pos_b[h].
     The last two terms are constant over k -> cancel in the softmax.  The
     remaining per-(k,h) term A enters as exp(logit+A) = exp(logit)*expA, and
     expA is folded into the value matrix and the softmax denominator.
  3. softmax denominator computed by a matmul with an expA-replicated
     stationary operand (per-head denominator broadcast over the 32 head
     channels for free); normalization is an approx-reciprocal + multiply.

Sharding: 8 cores = 4 batches x 2 query-halves.  Each core computes its
batch's cluster means (duplicated across the half-pair) and attention +
projection for its 4096 queries.  Output slices are disjoint.

PSUM discipline: matmul start=True clears has_written at bank granularity, so
accumulation groups never share a bank with another in-flight group
(separate tiles for the 3 means accumulators; j-outer/kc-inner ordering for
the col-tiled attend/denominator groups).
"""

import os
import numpy as np
import ml_dtypes
from contextlib import ExitStack

import concourse.bass as bass
import concourse.tile as tile
from concourse import bacc, mybir
from concourse.bass_utils import run_bass_kernel_spmd
from concourse.masks import make_identity

F16 = mybir.dt.float16
F32 = mybir.dt.float32
F8 = mybir.dt.float8e4

B, N, C, H, D, K, M = 4, 8192, 384, 12, 2, 256, 32
CH = C // H          # 32
NH = N // 2          # 4096 queries per core
G = 3                # head groups of 4 (row/col tiling)
NCK = 8              # n chunks of 512
NCHUNK = 512
NT = N // 128        # 64 feat row tiles
SCALE = CH ** -0.5


def _build_nc():
    nc = bacc.Bacc("TRN2", target_bir_lowering=False, debug=False)
    t = {}
    t["feat16"] = nc.dram_tensor("feat16", [N, C], F16, kind="ExternalInput")
    t["featq16"] = nc.dram_tensor("featq16", [NH, C], F16, kind="ExternalInput")
    t["s"] = nc.dram_tensor("s", [N, K], F8, kind="ExternalInput")
    t["expa"] = nc.dram_tensor("expa", [K, C], F16, kind="ExternalInput")
    t["wqT"] = nc.dram_tensor("wqT", [C, C], F16, kind="ExternalInput")
    t["wkT"] = nc.dram_tensor("wkT", [C, C], F16, kind="ExternalInput")
    t["wvT"] = nc.dram_tensor("wvT", [C, C], F16, kind="ExternalInput")
    t["wpT"] = nc.dram_tensor("wpT", [C, C], F16, kind="ExternalInput")
    t["bq"] = nc.dram_tensor("bq", [128, G], F32, kind="ExternalInput")
    t["bk"] = nc.dram_tensor("bk", [128, G], F32, kind="ExternalInput")
    t["bv"] = nc.dram_tensor("bv", [1, C], F16, kind="ExternalInput")
    t["bp"] = nc.dram_tensor("bp", [C], F32, kind="ExternalInput")
    t["out"] = nc.dram_tensor("out", [NH, C], F32, kind="ExternalOutput")
    _emit(nc, t)
    nc.compile()
    return nc


def _emit(nc, t):
    with tile.TileContext(nc) as tc, ExitStack() as ctx:
        consts = ctx.enter_context(tc.tile_pool(name="consts", bufs=1))
        big = ctx.enter_context(tc.tile_pool(name="big", bufs=1))
        work = ctx.enter_context(tc.tile_pool(name="work", bufs=3))

        # ---- weights needed early ------------------------------------------------
        w_sb = {}
        for w in ("wkT", "wqT"):
            w_sb[w] = consts.tile([128, G, C], F16, name=w + "_sb")
            nc.sync.dma_start(
                w_sb[w], t[w].ap().rearrange("(ci p) co -> p ci co", p=128)
            )
        ident = consts.tile([128, 128], F16)
        make_identity(nc, ident)
        ones1 = consts.tile([1, 128], F16)
        nc.vector.memset(ones1, 1.0)

        # ---- big persistent SBUF tensors ----------------------------------------
        # (p t) layout: partition p holds contiguous DRAM rows p*64+t -> one
        # large descriptor per partition per DMA.  Valid because the means
        # contraction pairs S and feat rows positionally; any row->partition
        # assignment works as long as both tensors use the same one.
        featv = t["feat16"].ap().rearrange("(p t) c -> p t c", p=128)
        sv = t["s"].ap().rearrange("(p t) k -> p t k", p=128)
        featT_sb = big.tile([128, G, NH], F16)
        qT_sb = big.tile([128, G, NH], F16)
        outnT_sb = big.tile([128, G, NH], F16)
        fm_nat = big.tile([128, 2, C], F16)   # feat cluster means, natural [k, c]
        fmT_sb = big.tile([128, G, K], F16)   # feat cluster means, transposed
        keyT_sb = big.tile([128, G, K], F16)
        vsc_sb = big.tile([128, 2, C], F16)   # v * expA, natural [k, c]

        # ---- phase 1: cluster sums (S-stationary matmul), key/value means -------
        ph1 = tc.alloc_tile_pool(name="ph1", bufs=1)
        feat_sb = ph1.tile([128, NT, C], F16)
        s_sb = ph1.tile([128, NT, K], F8)
        with tc.tile_pool(name="ps_pre", bufs=1, space="PSUM") as ps_pre:
            mps = [
                ps_pre.tile([128, C], F32, tag=f"msum{kc}", name=f"mps{kc}")
                for kc in range(2)
            ]
            # first transposes early (cheap, unblock q(nc0/nc1) later)
            for nci in range(2):
                for g in range(G):
                    nc.scalar.dma_start_transpose(
                        featT_sb[:, g, nci * NCHUNK : (nci + 1) * NCHUNK],
                        t["featq16"].ap()[
                            nci * NCHUNK : (nci + 1) * NCHUNK,
                            g * 128 : (g + 1) * 128,
                        ],
                    )
            # progressive chunk sizes: small first chunk bootstraps the PE fast
            edges = [0, 4, 12, 24, 40, 64]
            for c in range(len(edges) - 1):
                sl = slice(edges[c], edges[c + 1])
                nc.sync.dma_start(feat_sb[:, sl, :], featv[:, sl, :])
                nc.scalar.dma_start(s_sb[:, sl, :], sv[:, sl, :])
            expa_rep = consts.tile([128, 2, C], F16)
            nc.scalar.dma_start(
                expa_rep, t["expa"].ap().rearrange("(kt p) c -> p kt c", p=128)
            )
            for w in ("wvT", "wpT"):
                w_sb[w] = consts.tile([128, G, C], F16, name=w + "_sb")
                nc.scalar.dma_start(
                    w_sb[w], t[w].ap().rearrange("(ci p) co -> p ci co", p=128)
                )
            bq_sb = consts.tile([128, G], F32)
            nc.scalar.dma_start(bq_sb, t["bq"].ap())
            bk_sb = consts.tile([128, G], F32)
            nc.scalar.dma_start(bk_sb, t["bk"].ap())
            bv_sb = consts.tile([1, C], F16)
            nc.scalar.dma_start(bv_sb, t["bv"].ap())
            for i in range(NT):
                for kc in range(2):
                    nc.tensor.matmul(
                        mps[kc],
                        lhsT=s_sb[:, i, kc * 128 : (kc + 1) * 128],
                        rhs=feat_sb[:, i, :],
                        start=(i == 0),
                        stop=(i == NT - 1),
                    )
            # means: scale to f16, then transpose k,c -> c,k on the PE
            for kc in range(2):
                nc.vector.tensor_scalar_mul(fm_nat[:, kc, :], mps[kc], 1.0 / M)
            for kc in range(2):
                for g in range(G):
                    tp = ps_pre.tile([128, 128], F16, tag="kvps", bufs=2, name="tp")
                    nc.tensor.transpose(
                        tp, fm_nat[:, kc, g * 128 : (g + 1) * 128], ident
                    )
                    nc.vector.tensor_copy(
                        fmT_sb[:, g, kc * 128 : (kc + 1) * 128], tp
                    )
            # keyT = Wk @ fmT (+bk)
            for ct in range(G):
                kps = ps_pre.tile([128, K], F32, tag="kvps", bufs=2)
                for ci in range(G):
                    nc.tensor.matmul(
                        kps,
                        lhsT=w_sb["wkT"][:, ci, ct * 128 : (ct + 1) * 128],
                        rhs=fmT_sb[:, ci, :],
                        start=(ci == 0),
                        stop=(ci == G - 1),
                    )
                nc.vector.tensor_scalar_add(
                    keyT_sb[:, ct, :], kps, bk_sb[:, ct : ct + 1]
                )
            # v = fm @ Wv.T (+bv), then scaled by expA
            for kt in range(2):
                vps = ps_pre.tile([128, C], F32, tag="kvps", bufs=2)
                for ci in range(G):
                    nc.tensor.matmul(
                        vps,
                        lhsT=fmT_sb[:, ci, kt * 128 : (kt + 1) * 128],
                        rhs=w_sb["wvT"][:, ci, :],
                        start=(ci == 0),
                        stop=False,
                    )
                nc.tensor.matmul(vps, lhsT=ones1, rhs=bv_sb, start=False, stop=True)
                nc.vector.tensor_mul(vsc_sb[:, kt, :], vps, expa_rep[:, kt, :])
            del mps
        ph1.release()

        # ---- phase 2: q, attention, projection ----------------------------------
        with (
            tc.tile_pool(name="ptp_", bufs=12) as ptp,
            tc.tile_pool(name="ps_lg", bufs=2, space="PSUM") as ps_lg,
            tc.tile_pool(name="ps_sm", bufs=2, space="PSUM") as ps_sm,
        ):
            bp_sb = consts.tile([128, C], F32)
            nc.gpsimd.dma_start(
                bp_sb, bass.AP(tensor=t["bp"], offset=0, ap=[[0, 128], [1, C]])
            )

            def emit_q(nci):
                ns = nci * NCHUNK
                for g in range(G):
                    qps = ps_sm.tile([128, NCHUNK], F32, tag="ps512", name="qps")
                    for ci in range(G):
                        nc.tensor.matmul(
                            qps,
                            lhsT=w_sb["wqT"][:, ci, g * 128 : (g + 1) * 128],
                            rhs=featT_sb[:, ci, ns : ns + NCHUNK],
                            start=(ci == 0),
                            stop=(ci == G - 1),
                        )
                    nc.vector.tensor_scalar_add(
                        qT_sb[:, g, ns : ns + NCHUNK], qps, bq_sb[:, g : g + 1]
                    )

            def emit_proj(nci):
                for ti in range(NCHUNK // 128):
                    n0 = nci * NCHUNK + ti * 128
                    pps = ps_sm.tile([128, C], F32, tag="ps512", name="pps")
                    for ci in range(G):
                        nc.tensor.matmul(
                            pps,
                            lhsT=outnT_sb[:, ci, n0 : n0 + 128],
                            rhs=w_sb["wpT"][:, ci, :],
                            start=(ci == 0),
                            stop=(ci == G - 1),
                        )
                    ot = work.tile([128, C], F32, tag="ot")
                    nc.vector.tensor_add(ot, pps, bp_sb)
                    nc.sync.dma_start(t["out"].ap()[n0 : n0 + 128, :], ot)

            def emit_transpose(nci):
                for g in range(G):
                    nc.sync.dma_start_transpose(
                        featT_sb[:, g, nci * NCHUNK : (nci + 1) * NCHUNK],
                        t["featq16"].ap()[
                            nci * NCHUNK : (nci + 1) * NCHUNK,
                            g * 128 : (g + 1) * 128,
                        ],
                    )

            for nci in range(NCK):
                ns = nci * NCHUNK
                if nci + 2 < NCK:
                    emit_transpose(nci + 2)
                if nci == 0:
                    emit_q(0)
                # logits^T + exp, per contraction half (kc), 3-head PSUM tiles
                pts = {}
                for kc in range(2):
                    lts = [
                        ps_lg.tile([128, 3 * NCHUNK], F32, tag="lps", name="lt")
                        for _ in range(4)
                    ]
                    for g in range(G):
                        for j in range(4):
                            hh = 4 * g + j
                            tt, sl = hh // 3, hh % 3
                            nc.tensor.matmul(
                                lts[tt][:, sl * NCHUNK : (sl + 1) * NCHUNK],
                                lhsT=keyT_sb[
                                    j * 32 : (j + 1) * 32, g,
                                    kc * 128 : (kc + 1) * 128,
                                ],
                                rhs=qT_sb[j * 32 : (j + 1) * 32, g, ns : ns + NCHUNK],
                                start=True,
                                stop=True,
                                tile_position=(32 * j, 0),
                            )
                    for tt in range(4):
                        pt = ptp.tile([128, 3 * NCHUNK], F16, tag="pt", name="pt")
                        nc.scalar.activation(
                            pt, lts[tt], mybir.ActivationFunctionType.Exp
                        )
                        pts[(kc, tt)] = pt
                if nci + 1 < NCK:
                    emit_q(nci + 1)
                # attend + normalize; j outer / kc inner (bank-group safety)
                for g in range(G):
                    av = ps_sm.tile([128, NCHUNK], F32, tag="ps512", name="av")
                    dn = ps_sm.tile([128, NCHUNK], F32, tag="ps512", name="dn")
                    for j in range(4):
                        hh = 4 * g + j
                        tt, sl = hh // 3, hh % 3
                        for kc in range(2):
                            nc.tensor.matmul(
                                av[32 * j : 32 * (j + 1), :],
                                lhsT=vsc_sb[:, kc, hh * CH : (hh + 1) * CH],
                                rhs=pts[(kc, tt)][:, sl * NCHUNK : (sl + 1) * NCHUNK],
                                start=(kc == 0),
                                stop=(kc == 1),
                                tile_position=(0, 32 * j),
                            )
                        for kc in range(2):
                            nc.tensor.matmul(
                                dn[32 * j : 32 * (j + 1), :],
                                lhsT=expa_rep[:, kc, hh * CH : (hh + 1) * CH],
                                rhs=pts[(kc, tt)][:, sl * NCHUNK : (sl + 1) * NCHUNK],
                                start=(kc == 0),
                                stop=(kc == 1),
                                tile_position=(0, 32 * j),
                            )
                    rc = work.tile([128, NCHUNK], F32, tag="rc")
                    nc.vector.reciprocal_approx_fast(rc, dn)
                    nc.vector.tensor_mul(outnT_sb[:, g, ns : ns + NCHUNK], av, rc)
                if nci > 0:
                    emit_proj(nci - 1)
            emit_proj(NCK - 1)


_NC_CACHE = None


def kernel(pos, feat, member_idx, batch_idx, qkv_w, qkv_b, pos_w, pos_b,
           proj_w, proj_b, k):
    global _NC_CACHE
    pos = np.asarray(pos, np.float32)
    feat = np.asarray(feat, np.float32)
    member_idx = np.asarray(member_idx)
    qkv_w = np.asarray(qkv_w, np.float32)
    qkv_b = np.asarray(qkv_b, np.float32)
    pos_w = np.asarray(pos_w, np.float32)
    pos_b = np.asarray(pos_b, np.float32)
    proj_w = np.asarray(proj_w, np.float32)
    proj_b = np.asarray(proj_b, np.float32)

    # host-side input prep (sharding + index transforms + tiny pos branch)
    pos_n = pos / pos.reshape(-1, D).max(axis=0)
    feat16 = feat.astype(np.float16)

    wq = qkv_w[:C] * SCALE
    wqT = np.ascontiguousarray(wq.T).astype(np.float16)
    wkT = np.ascontiguousarray(qkv_w[C : 2 * C].T).astype(np.float16)
    wvT = np.ascontiguousarray(qkv_w[2 * C :].T).astype(np.float16)
    wpT = np.ascontiguousarray(proj_w.T).astype(np.float16)
    bq = np.ascontiguousarray((qkv_b[:C] * SCALE).reshape(G, 128).T).astype(np.float32)
    bk = np.ascontiguousarray(qkv_b[C : 2 * C].reshape(G, 128).T).astype(np.float32)
    bv = qkv_b[2 * C :].reshape(1, C).astype(np.float16)

    in_maps = []
    for b in range(B):
        mi = member_idx[b * K : (b + 1) * K]              # [K, M] row ids in batch
        S = np.zeros((N, K), ml_dtypes.float8_e4m3)
        S[mi.reshape(-1), np.repeat(np.arange(K), M)] = 1.0
        pm = pos_n[b][mi].mean(axis=1)                    # [K, D]
        expa = np.repeat(
            np.exp(pm @ pos_w.T), CH, axis=1
        ).astype(np.float16)                              # [K, H*CH]
        for half in range(2):
            in_maps.append(dict(
                feat16=feat16[b],
                featq16=feat16[b, half * NH : (half + 1) * NH],
                s=S, expa=expa,
                wqT=wqT, wkT=wkT, wvT=wvT, wpT=wpT,
                bq=bq, bk=bk, bv=bv, bp=proj_b,
            ))

    if _NC_CACHE is None:
        _NC_CACHE = _build_nc()
    nc = _NC_CACHE

    trace = bool(os.environ.get("KERNEL_TRACE"))
    if trace:
        _install_ntff_shim()
    res = run_bass_kernel_spmd(nc, in_maps, core_ids=list(range(8)), trace=trace)
    if trace:
        print("HW exec time:", res.exec_time_ns, "ns")
        if res.instructions_and_trace:
            print("trace:", res.instructions_and_trace[1])

    out = np.empty((B, N, C), np.float32)
    for b in range(B):
        for half in range(2):
            out[b, half * NH : (half + 1) * NH] = res.results[2 * b + half]["out"]
    return out


def _install_ntff_shim():
    import sys, types
    try:
        from antenv import axon_hooks  # noqa: F401
        return
    except ImportError:
        pass
    mod = types.ModuleType("antenv.axon_hooks")
    _hook = [None]
    mod.set_axon_ntff_profile_hook = lambda h: _hook.__setitem__(0, h)
    mod.get_axon_ntff_profile_hook = lambda: _hook[0]
    sys.modules["antenv.axon_hooks"] = mod
    import antenv
    antenv.axon_hooks = mod
    try:
        from trn_agent_boot.trn_boot import _ntff_profile_via_ctypes
        mod.set_axon_ntff_profile_hook(
            _ntff_profile_via_ctypes("/opt/axon/libaxon_pjrt.so")
        )
    except Exception as e:
        print("ntff shim failed:", e)


# revision 29
# speedup vs baseline: 1.0883x; 1.0883x over previous
"""ClusterAttention Trainium2 kernel.

Problem: B=4, N=8192, C=384, H=12, D=2, K=256 clusters of M=32 members.
  qkv = feat @ qkv_w.T + qkv_b
  kv/pos gathered per cluster -> mean -> key/value per (batch, cluster, head)
  attn = softmax(q.k*scale + pos_bias) over clusters; out = attn@v; proj.

Key algebraic restructurings:
  1. mean-of-gather commutes with the linear kv projection: cluster kv means
     are computed as (feat^T @ S) @ W_kv.T where S is the one-hot cluster
     assignment matrix -- no gather of the (much larger) kv tensor is needed.
  2. pos_bias[b,h,n,k] = pos_mean[b,k]@pos_w[h] - pos[b,n]@pos_w[h] + pos_b[h].
     The last two terms are constant over k -> cancel in the softmax.  The
     remaining per-(k,h) term A enters as exp(logit+A) = exp(logit)*expA, and
     expA is folded into the value matrix and the softmax denominator.
  3. softmax denominator computed by a matmul with an expA-replicated
     stationary operand (per-head denominator broadcast over the 32 head
     channels for free); normalization is an approx-reciprocal + multiply.

Sharding: 8 cores = 4 batches x 2 query-halves.  Each core computes its
batch's cluster means (duplicated across the half-pair) and attention +
projection for its 4096 queries.  Output slices are disjoint.

PSUM discipline: matmul start=True clears has_written at bank granularity, so
accumulation groups never share a bank with another in-flight group
(separate tiles for the 3 means accumulators; j-outer/kc-inner ordering for
the col-tiled attend/denominator groups).
"""

import os
import numpy as np
import ml_dtypes
from contextlib import ExitStack

import concourse.bass as bass
import concourse.tile as tile
from concourse import bacc, mybir
from concourse.bass_utils import run_bass_kernel_spmd
from concourse.masks import make_identity

F16 = mybir.dt.float16
F32 = mybir.dt.float32
F8 = mybir.dt.float8e4

B, N, C, H, D, K, M = 4, 8192, 384, 12, 2, 256, 32
CH = C // H          # 32
NH = N // 2          # 4096 queries per core
G = 3                # head groups of 4 (row/col tiling)
NCK = 8              # n chunks of 512
NCHUNK = 512
NT = N // 128        # 64 feat row tiles
SCALE = CH ** -0.5


def _build_nc():
    nc = bacc.Bacc("TRN2", target_bir_lowering=False, debug=False)
    t = {}
    t["feat16"] = nc.dram_tensor("feat16", [N, C], F16, kind="ExternalInput")
    t["featq16"] = nc.dram_tensor("featq16", [NH, C], F16, kind="ExternalInput")
    t["s"] = nc.dram_tensor("s", [N, K], F8, kind="ExternalInput")
    t["expa"] = nc.dram_tensor("expa", [K, C], F16, kind="ExternalInput")
    t["wqT"] = nc.dram_tensor("wqT", [C, C], F16, kind="ExternalInput")
    t["wkT"] = nc.dram_tensor("wkT", [C, C], F16, kind="ExternalInput")
    t["wvT"] = nc.dram_tensor("wvT", [C, C], F16, kind="ExternalInput")
    t["wpT"] = nc.dram_tensor("wpT", [C, C], F16, kind="ExternalInput")
    t["bq"] = nc.dram_tensor("bq", [128, G], F32, kind="ExternalInput")
    t["bk"] = nc.dram_tensor("bk", [128, G], F32, kind="ExternalInput")
    t["bv"] = nc.dram_tensor("bv", [1, C], F16, kind="ExternalInput")
    t["bp"] = nc.dram_tensor("bp", [C], F32, kind="ExternalInput")
    t["out"] = nc.dram_tensor("out", [NH, C], F32, kind="ExternalOutput")
    _emit(nc, t)
    nc.compile()
    return nc


def _emit(nc, t):
    with tile.TileContext(nc) as tc, ExitStack() as ctx:
        consts = ctx.enter_context(tc.tile_pool(name="consts", bufs=1))
        big = ctx.enter_context(tc.tile_pool(name="big", bufs=1))
        work = ctx.enter_context(tc.tile_pool(name="work", bufs=3))

        # ---- weights needed early ------------------------------------------------
        w_sb = {}
        for w in ("wkT", "wqT"):
            w_sb[w] = consts.tile([128, G, C], F16, name=w + "_sb")
            nc.sync.dma_start(
                w_sb[w], t[w].ap().rearrange("(ci p) co -> p ci co", p=128)
            )
        ident = consts.tile([128, 128], F16)
        make_identity(nc, ident)
        ones1 = consts.tile([1, 128], F16)
        nc.vector.memset(ones1, 1.0)

        # ---- big persistent SBUF tensors ----------------------------------------
        # (p t) layout: partition p holds contiguous DRAM rows p*64+t -> one
        # large descriptor per partition per DMA.  Valid because the means
        # contraction pairs S and feat rows positionally; any row->partition
        # assignment works as long as both tensors use the same one.
        featv = t["feat16"].ap().rearrange("(p t) c -> p t c", p=128)
        sv = t["s"].ap().rearrange("(p t) k -> p t k", p=128)
        featT_sb = big.tile([128, G, NH], F16)
        qT_sb = big.tile([128, G, NH], F16)
        outnT_sb = big.tile([128, G, NH], F16)
        fm_nat = big.tile([128, 2, C], F16)   # feat cluster means, natural [k, c]
        fmT_sb = big.tile([128, G, K], F16)   # feat cluster means, transposed
        keyT_sb = big.tile([128, G, K], F16)
        vsc_sb = big.tile([128, 2, C], F16)   # v * expA, natural [k, c]

        # ---- phase 1: cluster sums (S-stationary matmul), key/value means -------
        ph1 = tc.alloc_tile_pool(name="ph1", bufs=1)
        feat_sb = ph1.tile([128, NT, C], F16)
        s_sb = ph1.tile([128, NT, K], F8)
        with tc.tile_pool(name="ps_pre", bufs=1, space="PSUM") as ps_pre:
            mps = [
                ps_pre.tile([128, C], F32, tag=f"msum{kc}", name=f"mps{kc}")
                for kc in range(2)
            ]
            # progressive chunk sizes: small first chunk bootstraps the PE fast
            edges = [0, 4, 12, 24, 40, 64]
            for c in range(len(edges) - 1):
                sl = slice(edges[c], edges[c + 1])
                nc.sync.dma_start(feat_sb[:, sl, :], featv[:, sl, :])
                nc.scalar.dma_start(s_sb[:, sl, :], sv[:, sl, :])
            # query-half transposes (sync queue, after the feat loads)
            for g in range(G):
                nc.sync.dma_start_transpose(
                    featT_sb[:, g, :],
                    t["featq16"].ap()[:, g * 128 : (g + 1) * 128],
                )
            expa_rep = consts.tile([128, 2, C], F16)
            nc.scalar.dma_start(
                expa_rep, t["expa"].ap().rearrange("(kt p) c -> p kt c", p=128)
            )
            for w in ("wvT", "wpT"):
                w_sb[w] = consts.tile([128, G, C], F16, name=w + "_sb")
                nc.scalar.dma_start(
                    w_sb[w], t[w].ap().rearrange("(ci p) co -> p ci co", p=128)
                )
            bq_sb = consts.tile([128, G], F32)
            nc.scalar.dma_start(bq_sb, t["bq"].ap())
            bk_sb = consts.tile([128, G], F32)
            nc.scalar.dma_start(bk_sb, t["bk"].ap())
            bv_sb = consts.tile([1, C], F16)
            nc.scalar.dma_start(bv_sb, t["bv"].ap())
            for i in range(NT):
                for kc in range(2):
                    nc.tensor.matmul(
                        mps[kc],
                        lhsT=s_sb[:, i, kc * 128 : (kc + 1) * 128],
                        rhs=feat_sb[:, i, :],
                        start=(i == 0),
                        stop=(i == NT - 1),
                    )
            # means: scale to f16, then transpose k,c -> c,k on the PE
            for kc in range(2):
                nc.vector.tensor_scalar_mul(fm_nat[:, kc, :], mps[kc], 1.0 / M)
            for kc in range(2):
                for g in range(G):
                    tp = ps_pre.tile([128, 128], F16, tag="kvps", bufs=2, name="tp")
                    nc.tensor.transpose(
                        tp, fm_nat[:, kc, g * 128 : (g + 1) * 128], ident
                    )
                    nc.vector.tensor_copy(
                        fmT_sb[:, g, kc * 128 : (kc + 1) * 128], tp
                    )
            # keyT = Wk @ fmT (+bk)
            for ct in range(G):
                kps = ps_pre.tile([128, K], F32, tag="kvps", bufs=2)
                for ci in range(G):
                    nc.tensor.matmul(
                        kps,
                        lhsT=w_sb["wkT"][:, ci, ct * 128 : (ct + 1) * 128],
                        rhs=fmT_sb[:, ci, :],
                        start=(ci == 0),
                        stop=(ci == G - 1),
                    )
                nc.vector.tensor_scalar_add(
                    keyT_sb[:, ct, :], kps, bk_sb[:, ct : ct + 1]
                )
            # v = fm @ Wv.T (+bv), then scaled by expA
            for kt in range(2):
                vps = ps_pre.tile([128, C], F32, tag="kvps", bufs=2)
                for ci in range(G):
                    nc.tensor.matmul(
                        vps,
                        lhsT=fmT_sb[:, ci, kt * 128 : (kt + 1) * 128],
                        rhs=w_sb["wvT"][:, ci, :],
                        start=(ci == 0),
                        stop=False,
                    )
                nc.tensor.matmul(vps, lhsT=ones1, rhs=bv_sb, start=False, stop=True)
                nc.vector.tensor_mul(vsc_sb[:, kt, :], vps, expa_rep[:, kt, :])
            del mps
        ph1.release()

        # ---- phase 2: q, attention, projection ----------------------------------
        with (
            tc.tile_pool(name="ptp_", bufs=12) as ptp,
            tc.tile_pool(name="ps_lg", bufs=2, space="PSUM") as ps_lg,
            tc.tile_pool(name="ps_sm", bufs=2, space="PSUM") as ps_sm,
        ):
            bp_sb = consts.tile([128, C], F32)
            nc.gpsimd.dma_start(
                bp_sb, bass.AP(tensor=t["bp"], offset=0, ap=[[0, 128], [1, C]])
            )

            def emit_q(nci):
                ns = nci * NCHUNK
                for g in range(G):
                    qps = ps_sm.tile([128, NCHUNK], F32, tag="ps512", name="qps")
                    for ci in range(G):
                        nc.tensor.matmul(
                            qps,
                            lhsT=w_sb["wqT"][:, ci, g * 128 : (g + 1) * 128],
                            rhs=featT_sb[:, ci, ns : ns + NCHUNK],
                            start=(ci == 0),
                            stop=(ci == G - 1),
                        )
                    nc.vector.tensor_scalar_add(
                        qT_sb[:, g, ns : ns + NCHUNK], qps, bq_sb[:, g : g + 1]
                    )

            def emit_proj(nci):
                for ti in range(NCHUNK // 128):
                    n0 = nci * NCHUNK + ti * 128
                    pps = ps_sm.tile([128, C], F32, tag="ps512", name="pps")
                    for ci in range(G):
                        nc.tensor.matmul(
                            pps,
                            lhsT=outnT_sb[:, ci, n0 : n0 + 128],
                            rhs=w_sb["wpT"][:, ci, :],
                            start=(ci == 0),
                            stop=(ci == G - 1),
                        )
                    ot = work.tile([128, C], F32, tag="ot")
                    nc.vector.tensor_add(ot, pps, bp_sb)
                    nc.sync.dma_start(t["out"].ap()[n0 : n0 + 128, :], ot)

            for nci in range(NCK):
                ns = nci * NCHUNK
                if nci == 0:
                    emit_q(0)
                # logits^T + exp, per contraction half (kc), 3-head PSUM tiles
                pts = {}
                for kc in range(2):
                    lts = [
                        ps_lg.tile([128, 3 * NCHUNK], F32, tag="lps", name="lt")
                        for _ in range(4)
                    ]
                    for g in range(G):
                        for j in range(4):
                            hh = 4 * g + j
                            tt, sl = hh // 3, hh % 3
                            nc.tensor.matmul(
                                lts[tt][:, sl * NCHUNK : (sl + 1) * NCHUNK],
                                lhsT=keyT_sb[
                                    j * 32 : (j + 1) * 32, g,
                                    kc * 128 : (kc + 1) * 128,
                                ],
                                rhs=qT_sb[j * 32 : (j + 1) * 32, g, ns : ns + NCHUNK],
                                start=True,
                                stop=True,
                                tile_position=(32 * j, 0),
                            )
                    for tt in range(4):
                        pt = ptp.tile([128, 3 * NCHUNK], F16, tag="pt", name="pt")
                        nc.scalar.activation(
                            pt, lts[tt], mybir.ActivationFunctionType.Exp
                        )
                        pts[(kc, tt)] = pt
                if nci + 1 < NCK:
                    emit_q(nci + 1)
                # attend + normalize; j outer / kc inner (bank-group safety)
                for g in range(G):
                    av = ps_sm.tile([128, NCHUNK], F32, tag="ps512", name="av")
                    dn = ps_sm.tile([128, NCHUNK], F32, tag="ps512", name="dn")
                    for j in range(4):
                        hh = 4 * g + j
                        tt, sl = hh // 3, hh % 3
                        for kc in range(2):
                            nc.tensor.matmul(
                                av[32 * j : 32 * (j + 1), :],
                                lhsT=vsc_sb[:, kc, hh * CH : (hh + 1) * CH],
                                rhs=pts[(kc, tt)][:, sl * NCHUNK : (sl + 1) * NCHUNK],
                                start=(kc == 0),
                                stop=(kc == 1),
                                tile_position=(0, 32 * j),
                            )
                        for kc in range(2):
                            nc.tensor.matmul(
                                dn[32 * j : 32 * (j + 1), :],
                                lhsT=expa_rep[:, kc, hh * CH : (hh + 1) * CH],
                                rhs=pts[(kc, tt)][:, sl * NCHUNK : (sl + 1) * NCHUNK],
                                start=(kc == 0),
                                stop=(kc == 1),
                                tile_position=(0, 32 * j),
                            )
                    rc = work.tile([128, NCHUNK], F32, tag="rc")
                    nc.vector.reciprocal_approx_fast(rc, dn)
                    nc.vector.tensor_mul(outnT_sb[:, g, ns : ns + NCHUNK], av, rc)
                if nci > 0:
                    emit_proj(nci - 1)
            emit_proj(NCK - 1)


_NC_CACHE = None


def kernel(pos, feat, member_idx, batch_idx, qkv_w, qkv_b, pos_w, pos_b,
           proj_w, proj_b, k):
    global _NC_CACHE
    pos = np.asarray(pos, np.float32)
    feat = np.asarray(feat, np.float32)
    member_idx = np.asarray(member_idx)
    qkv_w = np.asarray(qkv_w, np.float32)
    qkv_b = np.asarray(qkv_b, np.float32)
    pos_w = np.asarray(pos_w, np.float32)
    pos_b = np.asarray(pos_b, np.float32)
    proj_w = np.asarray(proj_w, np.float32)
    proj_b = np.asarray(proj_b, np.float32)

    # host-side input prep (sharding + index transforms + tiny pos branch)
    pos_n = pos / pos.reshape(-1, D).max(axis=0)
    feat16 = feat.astype(np.float16)

    wq = qkv_w[:C] * SCALE
    wqT = np.ascontiguousarray(wq.T).astype(np.float16)
    wkT = np.ascontiguousarray(qkv_w[C : 2 * C].T).astype(np.float16)
    wvT = np.ascontiguousarray(qkv_w[2 * C :].T).astype(np.float16)
    wpT = np.ascontiguousarray(proj_w.T).astype(np.float16)
    bq = np.ascontiguousarray((qkv_b[:C] * SCALE).reshape(G, 128).T).astype(np.float32)
    bk = np.ascontiguousarray(qkv_b[C : 2 * C].reshape(G, 128).T).astype(np.float32)
    bv = qkv_b[2 * C :].reshape(1, C).astype(np.float16)

    in_maps = []
    for b in range(B):
        mi = member_idx[b * K : (b + 1) * K]              # [K, M] row ids in batch
        S = np.zeros((N, K), ml_dtypes.float8_e4m3)
        S[mi.reshape(-1), np.repeat(np.arange(K), M)] = 1.0
        pm = pos_n[b][mi].mean(axis=1)                    # [K, D]
        expa = np.repeat(
            np.exp(pm @ pos_w.T), CH, axis=1
        ).astype(np.float16)                              # [K, H*CH]
        for half in range(2):
            in_maps.append(dict(
                feat16=feat16[b],
                featq16=feat16[b, half * NH : (half + 1) * NH],
                s=S, expa=expa,
                wqT=wqT, wkT=wkT, wvT=wvT, wpT=wpT,
                bq=bq, bk=bk, bv=bv, bp=proj_b,
            ))

    if _NC_CACHE is None:
        _NC_CACHE = _build_nc()
    nc = _NC_CACHE

    trace = bool(os.environ.get("KERNEL_TRACE"))
    if trace:
        _install_ntff_shim()
    res = run_bass_kernel_spmd(nc, in_maps, core_ids=list(range(8)), trace=trace)
    if trace:
        print("HW exec time:", res.exec_time_ns, "ns")
        if res.instructions_and_trace:
            print("trace:", res.instructions_and_trace[1])

    out = np.empty((B, N, C), np.float32)
    for b in range(B):
        for half in range(2):
            out[b, half * NH : (half + 1) * NH] = res.results[2 * b + half]["out"]
    return out


def _install_ntff_shim():
    import sys, types
    try:
        from antenv import axon_hooks  # noqa: F401
        return
    except ImportError:
        pass
    mod = types.ModuleType("antenv.axon_hooks")
    _hook = [None]
    mod.set_axon_ntff_profile_hook = lambda h: _hook.__setitem__(0, h)
    mod.get_axon_ntff_profile_hook = lambda: _hook[0]
    sys.modules["antenv.axon_hooks"] = mod
    import antenv
    antenv.axon_hooks = mod
    try:
        from trn_agent_boot.trn_boot import _ntff_profile_via_ctypes
        mod.set_axon_ntff_profile_hook(
            _ntff_profile_via_ctypes("/opt/axon/libaxon_pjrt.so")
        )
    except Exception as e:
        print("ntff shim failed:", e)


# revision 30
# speedup vs baseline: 1.0981x; 1.0090x over previous
"""ClusterAttention Trainium2 kernel.

Problem: B=4, N=8192, C=384, H=12, D=2, K=256 clusters of M=32 members.
  qkv = feat @ qkv_w.T + qkv_b
  kv/pos gathered per cluster -> mean -> key/value per (batch, cluster, head)
  attn = softmax(q.k*scale + pos_bias) over clusters; out = attn@v; proj.

Key algebraic restructurings:
  1. mean-of-gather commutes with the linear kv projection: cluster kv means
     are computed as (feat^T @ S) @ W_kv.T where S is the one-hot cluster
     assignment matrix -- no gather of the (much larger) kv tensor is needed.
  2. pos_bias[b,h,n,k] = pos_mean[b,k]@pos_w[h] - pos[b,n]@pos_w[h] + pos_b[h].
     The last two terms are constant over k -> cancel in the softmax.  The
     remaining per-(k,h) term A enters as exp(logit+A) = exp(logit)*expA, and
     expA is folded into the value matrix and the softmax denominator.
  3. softmax denominator computed by a matmul with an expA-replicated
     stationary operand (per-head denominator broadcast over the 32 head
     channels for free); normalization is an approx-reciprocal + multiply.

Sharding: 8 cores = 4 batches x 2 query-halves.  Each core computes its
batch's cluster means (duplicated across the half-pair) and attention +
projection for its 4096 queries.  Output slices are disjoint.

PSUM discipline: matmul start=True clears has_written at bank granularity, so
accumulation groups never share a bank with another in-flight group
(separate tiles for the 3 means accumulators; j-outer/kc-inner ordering for
the col-tiled attend/denominator groups).
"""

import os
import numpy as np
import ml_dtypes
from contextlib import ExitStack

import concourse.bass as bass
import concourse.tile as tile
from concourse import bacc, mybir
from concourse.bass_utils import run_bass_kernel_spmd
from concourse.masks import make_identity

F16 = mybir.dt.float16
F32 = mybir.dt.float32
F8 = mybir.dt.float8e4

B, N, C, H, D, K, M = 4, 8192, 384, 12, 2, 256, 32
CH = C // H          # 32
NH = N // 2          # 4096 queries per core
G = 3                # head groups of 4 (row/col tiling)
NCK = 8              # n chunks of 512
NCHUNK = 512
NT = N // 128        # 64 feat row tiles
SCALE = CH ** -0.5


def _build_nc():
    nc = bacc.Bacc("TRN2", target_bir_lowering=False, debug=False)
    t = {}
    t["feat16"] = nc.dram_tensor("feat16", [N, C], F16, kind="ExternalInput")
    t["featq16"] = nc.dram_tensor("featq16", [NH, C], F16, kind="ExternalInput")
    t["s"] = nc.dram_tensor("s", [N, K], F8, kind="ExternalInput")
    t["expa"] = nc.dram_tensor("expa", [K, C], F16, kind="ExternalInput")
    t["wqT"] = nc.dram_tensor("wqT", [C, C], F16, kind="ExternalInput")
    t["wkT"] = nc.dram_tensor("wkT", [C, C], F16, kind="ExternalInput")
    t["wvT"] = nc.dram_tensor("wvT", [C, C], F16, kind="ExternalInput")
    t["wpT"] = nc.dram_tensor("wpT", [C, C], F16, kind="ExternalInput")
    t["bq"] = nc.dram_tensor("bq", [128, G], F32, kind="ExternalInput")
    t["bk"] = nc.dram_tensor("bk", [128, G], F32, kind="ExternalInput")
    t["bv"] = nc.dram_tensor("bv", [1, C], F16, kind="ExternalInput")
    t["bp"] = nc.dram_tensor("bp", [C], F32, kind="ExternalInput")
    t["out"] = nc.dram_tensor("out", [NH, C], F32, kind="ExternalOutput")
    _emit(nc, t)
    nc.compile()
    return nc


def _emit(nc, t):
    with tile.TileContext(nc) as tc, ExitStack() as ctx:
        consts = ctx.enter_context(tc.tile_pool(name="consts", bufs=1))
        big = ctx.enter_context(tc.tile_pool(name="big", bufs=1))
        work = ctx.enter_context(tc.tile_pool(name="work", bufs=3))

        # ---- weights needed early ------------------------------------------------
        w_sb = {}
        for w in ("wkT", "wqT"):
            w_sb[w] = consts.tile([128, G, C], F16, name=w + "_sb")
            nc.sync.dma_start(
                w_sb[w], t[w].ap().rearrange("(ci p) co -> p ci co", p=128)
            )
        ident = consts.tile([128, 128], F16)
        make_identity(nc, ident)
        ones1 = consts.tile([1, 128], F16)
        nc.vector.memset(ones1, 1.0)

        # ---- big persistent SBUF tensors ----------------------------------------
        # (p t) layout: partition p holds contiguous DRAM rows p*64+t -> one
        # large descriptor per partition per DMA.  Valid because the means
        # contraction pairs S and feat rows positionally; any row->partition
        # assignment works as long as both tensors use the same one.
        featv = t["feat16"].ap().rearrange("(p t) c -> p t c", p=128)
        sv = t["s"].ap().rearrange("(p t) k -> p t k", p=128)
        featT_sb = big.tile([128, G, NH], F16)
        qT_sb = big.tile([128, G, NH], F16)
        outnT_sb = big.tile([128, G, NH], F16)
        fm_nat = big.tile([128, 2, C], F16)   # feat cluster means, natural [k, c]
        fmT_sb = big.tile([128, G, K], F16)   # feat cluster means, transposed
        keyT_sb = big.tile([128, G, K], F16)
        vsc_sb = big.tile([128, 2, C], F16)   # v * expA, natural [k, c]

        # ---- phase 1: cluster sums (S-stationary matmul), key/value means -------
        ph1 = tc.alloc_tile_pool(name="ph1", bufs=1)
        feat_sb = ph1.tile([128, NT, C], F16)
        s_sb = ph1.tile([128, NT, K], F8)
        with tc.tile_pool(name="ps_pre", bufs=1, space="PSUM") as ps_pre:
            mps = [
                ps_pre.tile([128, C], F32, tag=f"msum{kc}", name=f"mps{kc}")
                for kc in range(2)
            ]
            edges = [0, 16, 32, 48, 64]
            for c in range(len(edges) - 1):
                sl = slice(edges[c], edges[c + 1])
                nc.sync.dma_start(feat_sb[:, sl, :], featv[:, sl, :])
                nc.scalar.dma_start(s_sb[:, sl, :], sv[:, sl, :])
            # query-half transposes (sync queue, after the feat loads)
            for g in range(G):
                nc.sync.dma_start_transpose(
                    featT_sb[:, g, :],
                    t["featq16"].ap()[:, g * 128 : (g + 1) * 128],
                )
            expa_rep = consts.tile([128, 2, C], F16)
            nc.scalar.dma_start(
                expa_rep, t["expa"].ap().rearrange("(kt p) c -> p kt c", p=128)
            )
            for w in ("wvT", "wpT"):
                w_sb[w] = consts.tile([128, G, C], F16, name=w + "_sb")
                nc.scalar.dma_start(
                    w_sb[w], t[w].ap().rearrange("(ci p) co -> p ci co", p=128)
                )
            bq_sb = consts.tile([128, G], F32)
            nc.scalar.dma_start(bq_sb, t["bq"].ap())
            bk_sb = consts.tile([128, G], F32)
            nc.scalar.dma_start(bk_sb, t["bk"].ap())
            bv_sb = consts.tile([1, C], F16)
            nc.scalar.dma_start(bv_sb, t["bv"].ap())
            for i in range(NT):
                for kc in range(2):
                    nc.tensor.matmul(
                        mps[kc],
                        lhsT=s_sb[:, i, kc * 128 : (kc + 1) * 128],
                        rhs=feat_sb[:, i, :],
                        start=(i == 0),
                        stop=(i == NT - 1),
                    )
            # means: scale to f16, then transpose k,c -> c,k on the PE
            for kc in range(2):
                nc.vector.tensor_scalar_mul(fm_nat[:, kc, :], mps[kc], 1.0 / M)
            for kc in range(2):
                for g in range(G):
                    tp = ps_pre.tile([128, 128], F16, tag="kvps", bufs=2, name="tp")
                    nc.tensor.transpose(
                        tp, fm_nat[:, kc, g * 128 : (g + 1) * 128], ident
                    )
                    nc.vector.tensor_copy(
                        fmT_sb[:, g, kc * 128 : (kc + 1) * 128], tp
                    )
            # keyT = Wk @ fmT (+bk)
            for ct in range(G):
                kps = ps_pre.tile([128, K], F32, tag="kvps", bufs=2)
                for ci in range(G):
                    nc.tensor.matmul(
                        kps,
                        lhsT=w_sb["wkT"][:, ci, ct * 128 : (ct + 1) * 128],
                        rhs=fmT_sb[:, ci, :],
                        start=(ci == 0),
                        stop=(ci == G - 1),
                    )
                nc.vector.tensor_scalar_add(
                    keyT_sb[:, ct, :], kps, bk_sb[:, ct : ct + 1]
                )
            # v = fm @ Wv.T (+bv), then scaled by expA
            for kt in range(2):
                vps = ps_pre.tile([128, C], F32, tag="kvps", bufs=2)
                for ci in range(G):
                    nc.tensor.matmul(
                        vps,
                        lhsT=fmT_sb[:, ci, kt * 128 : (kt + 1) * 128],
                        rhs=w_sb["wvT"][:, ci, :],
                        start=(ci == 0),
                        stop=False,
                    )
                nc.tensor.matmul(vps, lhsT=ones1, rhs=bv_sb, start=False, stop=True)
                nc.vector.tensor_mul(vsc_sb[:, kt, :], vps, expa_rep[:, kt, :])
            del mps
        ph1.release()

        # ---- phase 2: q, attention, projection ----------------------------------
        with (
            tc.tile_pool(name="ptp_", bufs=12) as ptp,
            tc.tile_pool(name="ps_lg", bufs=2, space="PSUM") as ps_lg,
            tc.tile_pool(name="ps_sm", bufs=2, space="PSUM") as ps_sm,
        ):
            bp_sb = consts.tile([128, C], F32)
            nc.gpsimd.dma_start(
                bp_sb, bass.AP(tensor=t["bp"], offset=0, ap=[[0, 128], [1, C]])
            )

            def emit_q(nci):
                ns = nci * NCHUNK
                for g in range(G):
                    qps = ps_sm.tile([128, NCHUNK], F32, tag="ps512", name="qps")
                    for ci in range(G):
                        nc.tensor.matmul(
                            qps,
                            lhsT=w_sb["wqT"][:, ci, g * 128 : (g + 1) * 128],
                            rhs=featT_sb[:, ci, ns : ns + NCHUNK],
                            start=(ci == 0),
                            stop=(ci == G - 1),
                        )
                    nc.vector.tensor_scalar_add(
                        qT_sb[:, g, ns : ns + NCHUNK], qps, bq_sb[:, g : g + 1]
                    )

            def emit_proj(nci):
                for ti in range(NCHUNK // 128):
                    n0 = nci * NCHUNK + ti * 128
                    pps = ps_sm.tile([128, C], F32, tag="ps512", name="pps")
                    for ci in range(G):
                        nc.tensor.matmul(
                            pps,
                            lhsT=outnT_sb[:, ci, n0 : n0 + 128],
                            rhs=w_sb["wpT"][:, ci, :],
                            start=(ci == 0),
                            stop=(ci == G - 1),
                        )
                    ot = work.tile([128, C], F32, tag="ot")
                    nc.vector.tensor_add(ot, pps, bp_sb)
                    nc.sync.dma_start(t["out"].ap()[n0 : n0 + 128, :], ot)

            for nci in range(NCK):
                ns = nci * NCHUNK
                if nci == 0:
                    emit_q(0)
                # logits^T + exp, per contraction half (kc), 3-head PSUM tiles
                pts = {}
                for kc in range(2):
                    lts = [
                        ps_lg.tile([128, 3 * NCHUNK], F32, tag="lps", name="lt")
                        for _ in range(4)
                    ]
                    for g in range(G):
                        for j in range(4):
                            hh = 4 * g + j
                            tt, sl = hh // 3, hh % 3
                            nc.tensor.matmul(
                                lts[tt][:, sl * NCHUNK : (sl + 1) * NCHUNK],
                                lhsT=keyT_sb[
                                    j * 32 : (j + 1) * 32, g,
                                    kc * 128 : (kc + 1) * 128,
                                ],
                                rhs=qT_sb[j * 32 : (j + 1) * 32, g, ns : ns + NCHUNK],
                                start=True,
                                stop=True,
                                tile_position=(32 * j, 0),
                            )
                    for tt in range(4):
                        pt = ptp.tile([128, 3 * NCHUNK], F16, tag="pt", name="pt")
                        nc.scalar.activation(
                            pt, lts[tt], mybir.ActivationFunctionType.Exp
                        )
                        pts[(kc, tt)] = pt
                if nci + 1 < NCK:
                    emit_q(nci + 1)
                # attend + normalize; j outer / kc inner (bank-group safety)
                for g in range(G):
                    av = ps_sm.tile([128, NCHUNK], F32, tag="ps512", name="av")
                    dn = ps_sm.tile([128, NCHUNK], F32, tag="ps512", name="dn")
                    for j in range(4):
                        hh = 4 * g + j
                        tt, sl = hh // 3, hh % 3
                        for kc in range(2):
                            nc.tensor.matmul(
                                av[32 * j : 32 * (j + 1), :],
                                lhsT=vsc_sb[:, kc, hh * CH : (hh + 1) * CH],
                                rhs=pts[(kc, tt)][:, sl * NCHUNK : (sl + 1) * NCHUNK],
                                start=(kc == 0),
                                stop=(kc == 1),
                                tile_position=(0, 32 * j),
                            )
                        for kc in range(2):
                            nc.tensor.matmul(
                                dn[32 * j : 32 * (j + 1), :],
                                lhsT=expa_rep[:, kc, hh * CH : (hh + 1) * CH],
                                rhs=pts[(kc, tt)][:, sl * NCHUNK : (sl + 1) * NCHUNK],
                                start=(kc == 0),
                                stop=(kc == 1),
                                tile_position=(0, 32 * j),
                            )
                    rc = work.tile([128, NCHUNK], F32, tag="rc")
                    nc.vector.reciprocal_approx_fast(rc, dn)
                    nc.vector.tensor_mul(outnT_sb[:, g, ns : ns + NCHUNK], av, rc)
                if nci > 0:
                    emit_proj(nci - 1)
            emit_proj(NCK - 1)


_NC_CACHE = None


def kernel(pos, feat, member_idx, batch_idx, qkv_w, qkv_b, pos_w, pos_b,
           proj_w, proj_b, k):
    global _NC_CACHE
    pos = np.asarray(pos, np.float32)
    feat = np.asarray(feat, np.float32)
    member_idx = np.asarray(member_idx)
    qkv_w = np.asarray(qkv_w, np.float32)
    qkv_b = np.asarray(qkv_b, np.float32)
    pos_w = np.asarray(pos_w, np.float32)
    pos_b = np.asarray(pos_b, np.float32)
    proj_w = np.asarray(proj_w, np.float32)
    proj_b = np.asarray(proj_b, np.float32)

    # host-side input prep (sharding + index transforms + tiny pos branch)
    pos_n = pos / pos.reshape(-1, D).max(axis=0)
    feat16 = feat.astype(np.float16)

    wq = qkv_w[:C] * SCALE
    wqT = np.ascontiguousarray(wq.T).astype(np.float16)
    wkT = np.ascontiguousarray(qkv_w[C : 2 * C].T).astype(np.float16)
    wvT = np.ascontiguousarray(qkv_w[2 * C :].T).astype(np.float16)
    wpT = np.ascontiguousarray(proj_w.T).astype(np.float16)
    bq = np.ascontiguousarray((qkv_b[:C] * SCALE).reshape(G, 128).T).astype(np.float32)
    bk = np.ascontiguousarray(qkv_b[C : 2 * C].reshape(G, 128).T).astype(np.float32)
    bv = qkv_b[2 * C :].reshape(1, C).astype(np.float16)

    in_maps = []
    for b in range(B):
        mi = member_idx[b * K : (b + 1) * K]              # [K, M] row ids in batch
        S = np.zeros((N, K), ml_dtypes.float8_e4m3)
        S[mi.reshape(-1), np.repeat(np.arange(K), M)] = 1.0
        pm = pos_n[b][mi].mean(axis=1)                    # [K, D]
        expa = np.repeat(
            np.exp(pm @ pos_w.T), CH, axis=1
        ).astype(np.float16)                              # [K, H*CH]
        for half in range(2):
            in_maps.append(dict(
                feat16=feat16[b],
                featq16=feat16[b, half * NH : (half + 1) * NH],
                s=S, expa=expa,
                wqT=wqT, wkT=wkT, wvT=wvT, wpT=wpT,
                bq=bq, bk=bk, bv=bv, bp=proj_b,
            ))

    if _NC_CACHE is None:
        _NC_CACHE = _build_nc()
    nc = _NC_CACHE

    trace = bool(os.environ.get("KERNEL_TRACE"))
    if trace:
        _install_ntff_shim()
    res = run_bass_kernel_spmd(nc, in_maps, core_ids=list(range(8)), trace=trace)
    if trace:
        print("HW exec time:", res.exec_time_ns, "ns")
        if res.instructions_and_trace:
            print("trace:", res.instructions_and_trace[1])

    out = np.empty((B, N, C), np.float32)
    for b in range(B):
        for half in range(2):
            out[b, half * NH : (half + 1) * NH] = res.results[2 * b + half]["out"]
    return out


def _install_ntff_shim():
    import sys, types
    try:
        from antenv import axon_hooks  # noqa: F401
        return
    except ImportError:
        pass
    mod = types.ModuleType("antenv.axon_hooks")
    _hook = [None]
    mod.set_axon_ntff_profile_hook = lambda h: _hook.__setitem__(0, h)
    mod.get_axon_ntff_profile_hook = lambda: _hook[0]
    sys.modules["antenv.axon_hooks"] = mod
    import antenv
    antenv.axon_hooks = mod
    try:
        from trn_agent_boot.trn_boot import _ntff_profile_via_ctypes
        mod.set_axon_ntff_profile_hook(
            _ntff_profile_via_ctypes("/opt/axon/libaxon_pjrt.so")
        )
    except Exception as e:
        print("ntff shim failed:", e)
